# revision 1
# baseline (speedup 1.0000x reference)
"""CKAFormer Trainium2 kernel, fp8 edition.

6 iterations of
    Xn = X / ||X||_row;  P = softmax(relu(Xn@W1+b1)@W2+b2)
    X  = Xn + g*P@(P.T@Xn) - g*Xn@(Xn.T@Xn)
then a final MLP. Row-sharded over 8 NeuronCores.

Speed scheme vs the bf16 baseline:
- State is kept per-row-SCALED (S = nrm*X): the row normalization of the
  leading term cancels. True X is recovered by one in-place scale pass in
  the last iteration only.
- U and V updates accumulate into ONE PSUM bank: with g8 = -8*G,
  er8 = 16*P, ptx8 = 8*PtX and xt8 = 16*Xn^T, both terms come out as
  128*(P@PtX - Xn@G), so a single scalar_tensor_tensor with per-row
  scalar sd*GAMMA/128 applies the whole update.
- All big in-loop matmuls run fp8e4m3 in DoubleRow perf mode (2 k-blocks
  contracted per pass): Gram, U, PtX, MLP1. Transposes and V run plain
  fp8 (fp8 transposes write PSUM at element step 2 per hw requirement).
- G is AllReduced in fp8 as top-right + bottom-right + top-left [512,512] chunks;
  the bottom-left quadrant is reconstructed locally as
  transpose(top-right). PtX is a third fp8 AllReduce. Update error enters
  X only through GAMMA=1e-4, so fp8 noise is ~2e-5 in the final output.
- Element-wise work is split across DVE and ACT only (GpSimd ucode tensor
  ops are ~10x slower - measured - and GpSimd cannot touch PSUM).
The final MLP also runs fp8 (error 1.6e-3 vs the 2e-2 gate), reusing
w18 and the transpose path on the last iteration's output.
"""

import sys

sys.path.insert(0, "/opt/trn_rl_repo")

import math

import ml_dtypes
import numpy as np

import concourse.bass as bass
import concourse.mybir as mybir
import concourse.tile as tile
from concourse.bass_utils import run_bass_kernel_spmd
from concourse.masks import make_identity
from concourse.vector_clock import ScopedClock

DEPTH = 6
GAMMA = 1e-4
DIM = 1024
HIDDEN = 16
OUT_DIM = 64
N = 16384
CORES = 8

NS = N // CORES        # rows per core = 2048
RT = NS // 128         # row tiles = 16
DK = DIM // 128        # dim k-tiles = 8
P = 128

F32 = mybir.dt.float32
F32R = mybir.dt.float32r
BF = mybir.dt.bfloat16
F8 = mybir.dt.float8e4
AF = mybir.ActivationFunctionType
ALU = mybir.AluOpType
DR = mybir.MatmulPerfMode.DoubleRow

SX = 16.0    # xb8/xt8 = SX * Xn
SW1 = 32.0   # w18 = SW1 * W1
SP = 16.0    # er8/er8t = SP * P
SE = 4.0     # et8 = SE * E
SG = 8.0     # g8 wire = -SG * G ;  ptx8 wire = SG * PtX

GRAM_DRAIN = -SG / (SX * SX)           # psum(SX^2 G) -> -8*G
PTX_DRAIN = SG / (SP * SX)             # psum(SP*SX*PtX) -> 8*PtX
MLP1_SCALE = 1.0 / (SX * SW1)          # psum -> Xn@W1
CUV = GAMMA / (SX * SG)                # fused STT: svc = sd*GAMMA/128

# this container's walrus only accepts one sync-wait slot per engine
# instruction; hoist excess waits onto preceding EventSemaphore carriers.
_MAX_WAITS = 1


class _TC(tile.TileContext):
    def _drain_and_barrier(self, tick_clock, wait_clock):
        drain_inst = self.nc.sync.drain()
        wait_clock.add_sem_waits(
            drain_inst.ins, ScopedClock({None: tick_clock.global_clock})
        )
        si = drain_inst.ins.sync_info
        w = list(si.on_wait) if si and si.on_wait else []
        if len(w) > 1:
            si.on_wait = w[:1]
            for i in range(1, len(w)):
                c = self.nc.sync.drain()
                c.ins.sync_info = mybir.SyncInfo(on_wait=[w[i]], on_update=[])
        self.nc.all_engine_barrier()
        assert self.sems is not None
        popped = self.nc._tile_sem_poison_stack.pop()
        assert popped is self._sem_poison
        self.nc.clear_and_free_semaphores(list(self.sems.allocated().values()))
        self.nc.all_engine_barrier()


def _split_waits(nc, limit=_MAX_WAITS):
    """Hoist excess sem waits onto EventSemaphore carriers inserted just
    before the over-limit instruction (per-engine program order preserves the
    gating; waits are a conjunction so splitting is sound)."""
    nid = 0
    for bb in nc.main_func.blocks:
        out = []
        changed = False
        for ins in bb.instructions:
            si = ins.sync_info
            w = list(si.on_wait) if si and si.on_wait else []
            if len(w) > limit:
                extra, keep = w[:-limit], w[-limit:]
                for i in range(0, len(extra), limit):
                    ev = mybir.InstEventSemaphore(name=f"wsplit_{nid}", ins=[], outs=[])
                    nid += 1
                    ev.engine = ins.engine
                    ev.sync_info = mybir.SyncInfo(
                        on_wait=extra[i : i + limit], on_update=[]
                    )
                    out.append(ev)
                si.on_wait = keep
                changed = True
            out.append(ins)
        if changed:
            bb.instructions = out


def _build():
    nc = bass.Bass()
    x_ext = nc.declare_dram_parameter("x", [NS, DIM], F32, isOutput=False)
    w18_ext = nc.declare_dram_parameter("w18", [P, DK * HIDDEN], F8, isOutput=False)
    w2b_ext = nc.declare_dram_parameter("w2b", [HIDDEN, OUT_DIM], BF, isOutput=False)
    b1_ext = nc.declare_dram_parameter("b1", [HIDDEN, 1], F32, isOutput=False)
    b2_ext = nc.declare_dram_parameter("b2", [OUT_DIM, 1], F32, isOutput=False)
    b2e_ext = nc.declare_dram_parameter("b2e", [OUT_DIM, 1], F32, isOutput=False)
    y_ext = nc.declare_dram_parameter("y", [NS, OUT_DIM], F32, isOutput=True)

    with _TC(nc) as tc:
        with (
            tc.tile_pool(name="state", bufs=1) as st,
            tc.tile_pool(name="sq", bufs=1) as sqp,
            tc.tile_pool(name="stg", bufs=6) as stg,
            tc.tile_pool(name="xtmp", bufs=1) as xtp,
            tc.tile_pool(name="ps", bufs=8, space="PSUM") as ps,
            tc.tile_pool(name="dram", bufs=2, space="DRAM") as dram,
        ):
            # persistent state
            xr = [st.tile([P, DIM], F32, name=f"xr{i}", tag=f"xr{i}") for i in range(RT)]
            xb8 = [st.tile([P, 2, DIM], F8, name=f"xb8{j}", tag=f"xb8{j}") for j in range(DK)]
            xt8 = [st.tile([P, 2, NS], F8, name=f"xt8{k}", tag=f"xt8{k}") for k in range(DK // 2)]
            g8 = [st.tile([P, 2, DIM], F8, name=f"g8{k}", tag=f"g8{k}") for k in range(DK // 2)]
            et8 = st.tile([OUT_DIM, NS], F8, name="et8", tag="et8")
            er8 = st.tile([P, RT, OUT_DIM], F8, name="er8", tag="er8")
            er8t = st.tile([OUT_DIM, NS], F8, name="er8t", tag="er8t")
            ptx8 = st.tile([OUT_DIM, DIM], F8, name="ptx8", tag="ptx8")
            a1 = st.tile([HIDDEN, NS], BF, name="a1", tag="a1")
            w18 = st.tile([P, DK * HIDDEN], F8, name="w18", tag="w18")
            w2b = st.tile([HIDDEN, OUT_DIM], BF, name="w2b", tag="w2b")
            b1 = st.tile([HIDDEN, 1], F32, name="b1", tag="b1")
            b2 = st.tile([OUT_DIM, 1], F32, name="b2", tag="b2")
            b2e = st.tile([OUT_DIM, 1], F32, name="b2e", tag="b2e")
            ident8 = st.tile([P, P], F8, name="ident8", tag="ident8")
            identf = st.tile([P, P], F32, name="identf", tag="identf")
            # per-iteration stats, double-buffered across iterations
            n2 = [st.tile([P, RT], F32, name=f"n2{s}", tag=f"n2{s}") for s in range(2)]
            sd = [st.tile([P, RT], F32, name=f"sd{s}", tag=f"sd{s}") for s in range(2)]
            inv = [st.tile([P, RT], F32, name=f"inv{s}", tag=f"inv{s}") for s in range(2)]
            sxv = [st.tile([P, RT], F32, name=f"sxv{s}", tag=f"sxv{s}") for s in range(2)]
            srow = [st.tile([P, RT], F32, name=f"srow{s}", tag=f"srow{s}") for s in range(2)]
            s16 = [st.tile([P, RT], F32, name=f"s16{s}", tag=f"s16{s}") for s in range(2)]
            svc = [st.tile([P, RT], F32, name=f"svc{s}", tag=f"svc{s}") for s in range(2)]

            # loads
            for i in range(RT):
                nc.sync.dma_start(xr[i][:], x_ext[i * P : (i + 1) * P, :])
            nc.sync.dma_start(w18[:], w18_ext[:, :])
            nc.sync.dma_start(w2b[:], w2b_ext[:, :])
            nc.sync.dma_start(b1[:], b1_ext[:, :])
            nc.sync.dma_start(b2[:], b2_ext[:, :])
            nc.sync.dma_start(b2e[:], b2e_ext[:, :])
            make_identity(nc, identf[:])
            nc.vector.tensor_copy(ident8[:], identf[:])

            def norm_block(i, s):
                # row norm stats of (raw) block i into stats set s;
                # then xb8 <- fp8(SX * Xn) in row-pair layout. Engines
                # alternate per block to split the load ACT/DVE.
                sq = sqp.tile([P, DIM], F32, name="sq", tag="sq")
                nc.scalar.activation(
                    sq[:], xr[i][:], AF.Square, accum_out=n2[s][:, i : i + 1]
                )
                nc.scalar.sqrt(sd[s][:, i : i + 1], n2[s][:, i : i + 1])
                nc.vector.reciprocal(inv[s][:, i : i + 1], sd[s][:, i : i + 1])
                nc.vector.tensor_scalar_mul(
                    sxv[s][:, i : i + 1], inv[s][:, i : i + 1], SX
                )
                dst = xb8[i // 2][:, i % 2, :]
                if i % 2 == 0:
                    nc.vector.tensor_scalar_mul(dst, xr[i][:], sxv[s][:, i : i + 1])
                else:
                    nc.scalar.activation(
                        dst, xr[i][:], AF.Copy, scale=sxv[s][:, i : i + 1]
                    )

            def phase_gram(ms, h, arin, drain_rr, row0=0):
                # partial (SX Xn).T @ (SX Xn) over row tiles for m-blocks `ms`,
                # column half h; drain scaled to -8*G fp8 into arin rows
                # (m-row0)*128.
                for m in ms:
                    pg = ps.tile([P, 512], F32, name="ps", tag="ps")
                    for j in range(DK):
                        nc.tensor.matmul(
                            pg[:],
                            xb8[j][:, :, m * P : (m + 1) * P],
                            xb8[j][:, :, h * 512 : (h + 1) * 512],
                            start=(j == 0),
                            stop=(j == DK - 1),
                            perf_mode=DR,
                        )
                    gs = stg.tile([P, 512], F8, name="gs", tag="gs")
                    if drain_rr.pop(0) == "a":
                        nc.scalar.mul(gs[:], pg[:], GRAM_DRAIN)
                    else:
                        nc.vector.tensor_scalar_mul(gs[:], pg[:], GRAM_DRAIN)
                    nc.sync.dma_start(arin[(m - row0) * P : (m - row0 + 1) * P, :], gs[:])

            def phase_transpose():
                # xt8[k//2][:, k%2, r] = fp8(SX*Xn[r, kblock].T) on PE.
                # 8 stride-2 transposes packed per PSUM bank, one big copy out.
                for k in range(DK):
                    for jg2 in range(2):
                        pt = ps.tile([P, 2048], F8, name="ps8", tag="ps")
                        for q in range(8):
                            i = 8 * jg2 + q
                            nc.tensor.transpose(
                                pt[:, q * 2 * P : (q + 1) * 2 * P : 2],
                                xb8[i // 2][:, i % 2, k * P : (k + 1) * P],
                                ident8[:],
                            )
                        dst = xt8[k // 2][:, k % 2, jg2 * 1024 : (jg2 + 1) * 1024]
                        if (k + jg2) % 2 == 0:
                            nc.scalar.copy(dst, pt[:, 0:2048:2])
                        else:
                            nc.vector.tensor_copy(dst, pt[:, 0:2048:2])

            def phase_mlp():
                # a1 = relu(Xn@W1 + b1).T ; et8 = fp8(4*exp(.@W2+b2))
                for q in range(NS // 512):
                    sl = slice(q * 512, (q + 1) * 512)
                    pa = ps.tile([HIDDEN, 512], F32, name="ps", tag="ps")
                    for kk in range(DK // 2):
                        nc.tensor.matmul(
                            pa[:],
                            w18[:, kk * 2 * HIDDEN : (kk + 1) * 2 * HIDDEN].rearrange(
                                "p (t h) -> p t h", t=2
                            ),
                            xt8[kk][:, :, sl],
                            start=(kk == 0),
                            stop=(kk == DK // 2 - 1),
                            perf_mode=DR,
                        )
                    nc.scalar.activation(
                        a1[:, sl], pa[:], AF.Relu, bias=b1[:], scale=MLP1_SCALE
                    )
                    pb = ps.tile([OUT_DIM, 512], F32, name="ps", tag="ps")
                    nc.tensor.matmul(pb[:], w2b[:], a1[:, sl])
                    nc.scalar.activation(et8[:, sl], pb[:], AF.Exp, bias=b2e[:])

            def phase_et_transpose(s):
                # transpose et8 (4E) to rows (stride-2 fp8); srow = sum(4E)
                pts = []
                for j2 in range(2):
                    pt = ps.tile([P, 1024], F8, name="ps8", tag="ps")
                    for q in range(8):
                        i = 8 * j2 + q
                        nc.tensor.transpose(
                            pt[:, q * 2 * OUT_DIM : (q + 1) * 2 * OUT_DIM : 2],
                            et8[:, i * P : (i + 1) * P],
                            ident8[:OUT_DIM, :OUT_DIM],
                        )
                    nc.vector.tensor_reduce(
                        srow[s][:, 8 * j2 : 8 * j2 + 8],
                        pt[:].rearrange("p (i o t) -> p i o t", o=OUT_DIM, t=2)[:, :, :, 0],
                        mybir.AxisListType.X,
                        ALU.add,
                    )
                    pts.append(pt)
                return pts

            def phase_p(pts, s):
                # er8[:, i, :] = fp8(SP * P-rows) = pt * s16;  svc for the STT
                nc.vector.reciprocal(s16[s][:], srow[s][:])
                nc.vector.tensor_scalar_mul(s16[s][:], s16[s][:], SP)
                nc.vector.tensor_scalar_mul(svc[s][:], sd[s][:], CUV)
                for j2 in range(2):
                    for q in range(8):
                        i = 8 * j2 + q
                        nc.vector.tensor_scalar_mul(
                            er8[:, i, :],
                            pts[j2][:, q * 2 * OUT_DIM : (q + 1) * 2 * OUT_DIM : 2],
                            s16[s][:, i : i + 1],
                        )

            def phase_er8t():
                # er8t = (16P).T via PE transposes of er8 rows
                for j2 in range(2):
                    pt = ps.tile([OUT_DIM, 2048], F8, name="pse", tag="ps")
                    for q in range(8):
                        i = 8 * j2 + q
                        nc.tensor.transpose(
                            pt[:, q * 2 * P : (q + 1) * 2 * P : 2],
                            er8[:, i, :],
                            ident8[:],
                        )
                    nc.vector.tensor_copy(
                        er8t[:, j2 * 1024 : (j2 + 1) * 1024], pt[:, 0:2048:2]
                    )

            def phase_ptx(arin):
                # partial (SP*P).T @ (SX*Xn) -> fp8(SG*PtX) wire
                for h in range(2):
                    pp = ps.tile([OUT_DIM, 512], F32, name="ps", tag="ps")
                    for j in range(DK):
                        nc.tensor.matmul(
                            pp[:],
                            er8[:, 2 * j : 2 * j + 2, :],
                            xb8[j][:, :, h * 512 : (h + 1) * 512],
                            start=(j == 0),
                            stop=(j == DK - 1),
                            perf_mode=DR,
                        )
                    pps = stg.tile([OUT_DIM, 512], F8, name="pps", tag="gs")
                    nc.scalar.mul(pps[:], pp[:], PTX_DRAIN)
                    nc.sync.dma_start(arin[:, h * 512 : (h + 1) * 512], pps[:])

            def phase_bl():
                # bottom-left of g8 = transpose(top-right): g8 cols 0:512 for
                # k-blocks 4..7 from g8 cols 512:1024 of k-blocks 0..3.
                for b in range(4):
                    pt = ps.tile([P, 1024], F8, name="ps8", tag="ps")
                    for a in range(4):
                        nc.tensor.transpose(
                            pt[:, a * 2 * P : (a + 1) * 2 * P : 2],
                            g8[a // 2][:, a % 2, 512 + b * P : 512 + (b + 1) * P],
                            ident8[:],
                        )
                    if b % 2 == 0:
                        nc.scalar.copy(g8[2 + b // 2][:, b % 2, 0:512], pt[:, 0:1024:2])
                    else:
                        nc.vector.tensor_copy(
                            g8[2 + b // 2][:, b % 2, 0:512], pt[:, 0:1024:2]
                        )

            def phase_uv(h, s, tail=None):
                # one fused PSUM chain per block: 128*(P@PtX - Xn@G) cols h,
                # then a single STT: xsl += svc * psum
                for i in range(RT):
                    xsl = xr[i][:, h * 512 : (h + 1) * 512]
                    pu = ps.tile([P, 512], F32, name="ps", tag="ps")
                    for kk in range(DK // 2):
                        nc.tensor.matmul(
                            pu[:],
                            xt8[kk][:, :, i * P : (i + 1) * P],
                            g8[kk][:, :, h * 512 : (h + 1) * 512],
                            start=(kk == 0),
                            stop=False,
                            perf_mode=DR,
                        )
                    nc.tensor.matmul(
                        pu[:],
                        er8t[:, i * P : (i + 1) * P],
                        ptx8[:, h * 512 : (h + 1) * 512],
                        start=False,
                        stop=True,
                    )
                    nc.vector.scalar_tensor_tensor(
                        xsl, pu[:], svc[s][:, i : i + 1], xsl, ALU.mult, ALU.add
                    )
                    if tail is not None:
                        tail(i)

            rg = [list(range(CORES))]
            for i in range(RT):
                norm_block(i, 0)

            for it in range(DEPTH):
                s = it % 2
                arin_tr = dram.tile([512, 512], F8, name="arin_tr", tag="arin_tr")
                arout_tr = dram.tile([512, 512], F8, name="arout_tr", tag="arout_tr", addr_space="Shared")
                arin_br = dram.tile([512, 512], F8, name="arin_br", tag="arin_br")
                arout_br = dram.tile([512, 512], F8, name="arout_br", tag="arout_br", addr_space="Shared")
                arin_tl = dram.tile([512, 512], F8, name="arin_tl", tag="arin_tl")
                arout_tl = dram.tile([512, 512], F8, name="arout_tl", tag="arout_tl", addr_space="Shared")
                arin_p = dram.tile([OUT_DIM, DIM], F8, name="arin_p", tag="arin_p")
                arout_p = dram.tile([OUT_DIM, DIM], F8, name="arout_p", tag="arout_p", addr_space="Shared")

                drains = list("vvvvvvvv" if it == 0 else "avavavav")
                phase_gram(range(DK // 2), 1, arin_tr, drains[:4], row0=0)
                nc.gpsimd.collective_compute(
                    "AllReduce", ALU.add,
                    ins=[arin_tr.opt()], outs=[arout_tr.opt()], replica_groups=rg,
                )
                phase_gram(range(DK // 2, DK), 1, arin_br, drains[4:], row0=DK // 2)
                nc.gpsimd.collective_compute(
                    "AllReduce", ALU.add,
                    ins=[arin_br.opt()], outs=[arout_br.opt()], replica_groups=rg,
                )
                phase_transpose()
                phase_mlp()
                pts = phase_et_transpose(s)
                phase_p(pts, s)
                phase_ptx(arin_p)
                nc.gpsimd.collective_compute(
                    "AllReduce", ALU.add,
                    ins=[arin_p.opt()], outs=[arout_p.opt()], replica_groups=rg,
                )
                phase_er8t()
                phase_gram(range(DK // 2), 0, arin_tl, list("avav"))
                nc.gpsimd.collective_compute(
                    "AllReduce", ALU.add,
                    ins=[arin_tl.opt()], outs=[arout_tl.opt()], replica_groups=rg,
                )
                # land AllReduce results
                for k in range(DK // 2):
                    nc.sync.dma_start(
                        g8[k // 2][:, k % 2, 512:1024],
                        arout_tr[k * P : (k + 1) * P, :],
                    )
                for k in range(DK // 2, DK):
                    nc.sync.dma_start(
                        g8[k // 2][:, k % 2, 512:1024],
                        arout_br[(k - DK // 2) * P : (k - DK // 2 + 1) * P, :],
                    )
                for k in range(DK // 2):
                    nc.sync.dma_start(
                        g8[k // 2][:, k % 2, 0:512],
                        arout_tl[k * P : (k + 1) * P, :],
                    )
                nc.sync.dma_start(ptx8[:, :], arout_p[:, :])
                phase_bl()
                phase_uv(1, s)
                if it < DEPTH - 1:
                    tail = lambda i: norm_block(i, (it + 1) % 2)
                else:
                    def tail(i):
                        # fb8 = fp8(SX * X_6) = fp8(S * SX*inv) into xb8 tiles
                        # (X_6 = S_6 * inv_5; xb8 is dead after this iter's
                        # gram/ptx so the tiles are recycled for the final MLP)
                        so = (it + 1) % 2
                        nc.vector.tensor_scalar_mul(
                            sxv[so][:, i : i + 1], inv[s][:, i : i + 1], SX
                        )
                        dst = xb8[i // 2][:, i % 2, :]
                        if i % 2 == 0:
                            nc.vector.tensor_scalar_mul(
                                dst, xr[i][:], sxv[so][:, i : i + 1]
                            )
                        else:
                            nc.scalar.activation(
                                dst, xr[i][:], AF.Copy,
                                scale=sxv[so][:, i : i + 1],
                            )
                phase_uv(0, s, tail=tail)

            # final MLP in fp8: transpose fb8 -> xt8, DR matmuls with w18
            phase_transpose()
            yt = xtp.tile([OUT_DIM, NS], F32, name="yt", tag="yt")
            for q in range(4):
                sl = slice(q * 512, (q + 1) * 512)
                pa = ps.tile([HIDDEN, 512], F32, name="ps", tag="ps")
                for kk in range(DK // 2):
                    nc.tensor.matmul(
                        pa[:],
                        w18[:, kk * 2 * HIDDEN : (kk + 1) * 2 * HIDDEN].rearrange(
                            "p (t h) -> p t h", t=2
                        ),
                        xt8[kk][:, :, sl],
                        start=(kk == 0),
                        stop=(kk == DK // 2 - 1),
                        perf_mode=DR,
                    )
                nc.scalar.activation(
                    a1[:, sl], pa[:], AF.Relu, bias=b1[:], scale=MLP1_SCALE
                )
                pb = ps.tile([OUT_DIM, 512], F32, name="ps", tag="ps")
                nc.tensor.matmul(pb[:], w2b[:], a1[:, sl])
                nc.scalar.activation(yt[:, sl], pb[:], AF.Identity, bias=b2[:])
            # transpose Y.T -> rows and store
            yr = sqp.tile([P, RT, OUT_DIM], F32, name="yr", tag="sq")
            for j2 in range(2):
                pt = ps.tile([P, 512], F32, name="ps", tag="ps")
                for q in range(8):
                    i = 8 * j2 + q
                    nc.tensor.transpose(
                        pt[:, q * OUT_DIM : (q + 1) * OUT_DIM],
                        yt[:, i * P : (i + 1) * P].bitcast(F32),
                        identf[:OUT_DIM, :OUT_DIM],
                    )
                nc.vector.tensor_copy(yr[:, 8 * j2 : 8 * j2 + 8, :], pt[:])
            nc.sync.dma_start(
                y_ext.rearrange("(i p) o -> p i o", p=P), yr[:, :, :]
            )

    _split_waits(nc)
    return nc


_NC = None


def _get_nc():
    global _NC
    if _NC is None:
        _NC = _build()
    return _NC


def _q8(x):
    return np.clip(x, -240.0, 240.0).astype(ml_dtypes.float8_e4m3)


def _in_maps(X, W1, b1, W2, b2):
    X = np.asarray(X, dtype=np.float32)
    W1 = np.asarray(W1, dtype=np.float32)
    b1c = np.asarray(b1, dtype=np.float32).reshape(HIDDEN, 1)
    W2 = np.asarray(W2, dtype=np.float32)
    b2c = np.asarray(b2, dtype=np.float32).reshape(OUT_DIM, 1)
    b2e = b2c + np.float32(math.log(SE))
    # w18: fp8(SW1*W1) packed [128, kk, t, h] flat
    w18 = np.zeros((P, DK * HIDDEN), np.float32)
    for kk in range(DK // 2):
        for t in range(2):
            blk = W1[(2 * kk + t) * P : (2 * kk + t + 1) * P, :] * SW1
            w18[:, kk * 2 * HIDDEN + t * HIDDEN : kk * 2 * HIDDEN + (t + 1) * HIDDEN] = blk
    w18 = _q8(w18)
    w2bb = W2.astype(ml_dtypes.bfloat16)
    return [
        {
            "x": np.ascontiguousarray(X[c * NS : (c + 1) * NS]),
            "w18": w18,
            "w2b": w2bb,
            "b1": b1c,
            "b2": b2c,
            "b2e": b2e,
        }
        for c in range(CORES)
    ]


def run(X, W1, b1, W2, b2, **kwargs):
    nc = _get_nc()
    res = run_bass_kernel_spmd(nc, _in_maps(X, W1, b1, W2, b2), list(range(CORES)), **kwargs)
    out = np.concatenate([res.results[c]["y"] for c in range(CORES)], axis=0)
    return out, res


def kernel(X, W1, b1, W2, b2):
    out, _ = run(X, W1, b1, W2, b2)
    return out

